# revision 11
# baseline (speedup 1.0000x reference)
"""Trainium2 Bass kernel for Nadaraya-Watson kernel regression (retrieval_knn).

Reference computation (per output dim d, independently):
    z_d = train_X @ W[d]          [N]
    x_d = x @ W[d]                [B]
    k[n,b] = exp(-alpha/2 (z_n - x_b)^2),  alpha = 1/h^2
    out[b,d] = sum_n Y_n k[n,b] / sum_n k[n,b]

Instead of materializing the [N, B] kernel matrix (~100M exps), use the
factorization exp(-a/2(z-x)^2) = e^{-a z^2/2} e^{-a x^2/2} e^{a z x} and a
truncated Taylor expansion of e^{a z x} = sum_k (a z)^k x^k / k!.  The
e^{-a x^2/2} factor cancels in the num/den ratio, so:

    num[b,d] = sum_k A[k,d] x_d[b]^k,  A[k,d] = (1/k!) sum_n Y_n u[n,d] (a z)^k
    den[b,d] = sum_k C[k,d] x_d[b]^k,  C[k,d] = (1/k!) sum_n     u[n,d] (a z)^k
    u[n,d]   = exp(-a z_{n,d}^2 / 2)

with K=16 this matches the fp32 reference to ~1e-6 (validated numerically;
max |a z x| ~ 6.6 over the data distribution).

Sharding: batch B=4096 split across 8 cores (512 queries each); train side
replicated.  Each core computes the full train-side moments redundantly
(cheap) and evaluates its own queries.  No collectives.
"""

import math
from contextlib import ExitStack

import numpy as np

import concourse.bass as bass
import concourse.tile as tile
from concourse import bacc, mybir
from concourse.bass_utils import run_bass_kernel_spmd

F32 = mybir.dt.float32

N_TRAIN = 8192
B = 4096
D_IN = 4
D_OUT = 3
N_CORES = 8
B_LOC = B // N_CORES          # 512 queries per core
NCH = N_TRAIN // 128          # 64 train chunks (partition dim)
CD = NCH * D_OUT              # 192  (c, d) columns
K_DEG = 16                    # Taylor degree
NK = K_DEG + 1                # 17 terms
KD = NK * D_OUT               # 51  (k, d) moment columns
QC = B_LOC // 128             # 4 query chunks
QCD = QC * D_OUT              # 12


def _emit(nc: bass.Bass):
    x_in = nc.declare_dram_parameter("xq", [B_LOC, D_IN], F32, isOutput=False)
    tx_in = nc.declare_dram_parameter("train_x", [N_TRAIN, D_IN], F32, isOutput=False)
    y_in = nc.declare_dram_parameter("yv", [N_TRAIN], F32, isOutput=False)
    wh_in = nc.declare_dram_parameter("whp", [1, D_OUT * D_IN + 1], F32, isOutput=False)
    tbl_in = nc.declare_dram_parameter("tbl", [KD, KD + 2], F32, isOutput=False)
    o_out = nc.declare_dram_parameter("out", [B_LOC, D_OUT], F32, isOutput=True)

    with tile.TileContext(nc) as tc, ExitStack() as ctx:
        sb = ctx.enter_context(tc.tile_pool(name="sb", bufs=1))
        ps = ctx.enter_context(tc.tile_pool(name="ps", bufs=1, space="PSUM"))

        # --- ACT table preload (overlaps with DMAs) ---
        warm = sb.tile([1, 1], F32)
        nc.vector.memset(warm[:], 0.0)
        nc.scalar.activation(warm[:], warm[:], mybir.ActivationFunctionType.Square)
        nc.scalar.activation(warm[:], warm[:], mybir.ActivationFunctionType.Exp)

        # --- input DMAs ---
        # train_X rows n = p*64 + c  ->  XT[p, c*4 + j]   (1KB contig/partition)
        XT = sb.tile([128, NCH * D_IN], F32)
        nc.sync.dma_start(XT[:], tx_in[:, :].rearrange("(p c) d -> p (c d)", p=128))

        # W flat + h in one packed [1, 13] row (single DMA -> single queue)
        wh_row = sb.tile([1, D_OUT * D_IN + 1], F32)
        nc.sync.dma_start(wh_row[:], wh_in[:, :])

        # queries: rows b = p*4 + c -> XQ[p, c*4 + j]
        XQ = sb.tile([128, QC * D_IN], F32)
        nc.gpsimd.dma_start(XQ[:], x_in[:, :].rearrange("(p c) d -> p (c d)", p=128))

        # Y interleaved with ones: YO[:, 2c] = Y[p*64+c], YO[:, 2c+1] = 1
        YO = sb.tile([128, 2 * NCH], F32)
        nc.vector.memset(YO[:], 1.0)
        nc.gpsimd.dma_start(YO[:].rearrange("p (c two) -> p c two", two=2)[:, :, 0:1],
                            y_in[:].rearrange("(p c) -> p c", p=128).unsqueeze(2))

        # const tables: [ 1/k! (2 cols) | identity (51 cols) ] one DMA
        tblT = sb.tile([KD, KD + 2], F32)
        nc.gpsimd.dma_start(tblT[:], tbl_in[:, :])
        ifac = tblT[:, 0:2]
        ident = sb.tile([KD, KD], F32)
        nc.vector.tensor_copy(ident[:], tblT[:, 2 : KD + 2])

        # --- broadcast W,h across partitions via ones-matmul ---
        ones_row = sb.tile([1, 128], F32)
        nc.vector.memset(ones_row[:], 1.0)
        whr2 = sb.tile([1, D_OUT * D_IN + 1], F32)
        nc.vector.tensor_copy(whr2[:], wh_row[:])
        ps_wh = ps.tile([128, D_OUT * D_IN + 1], F32)
        nc.tensor.matmul(ps_wh[:], ones_row[:], whr2[:], start=True, stop=True)
        Wb = sb.tile([128, D_OUT * D_IN + 1], F32)
        nc.scalar.copy(Wb[:], ps_wh[:])

        # --- alpha = 1/h^2 per-partition columns ---
        hcol = Wb[:, 12:13]
        h2 = sb.tile([128, 1], F32)
        nc.vector.tensor_mul(h2[:], hcol, hcol)
        acol = sb.tile([128, 1], F32)
        nc.vector.reciprocal(acol[:], h2[:])
        nah = sb.tile([128, 1], F32)      # -alpha/2
        nc.vector.tensor_scalar_mul(nah[:], acol[:], -0.5)

        # --- Z[p, c*3+d] = sum_j XT[p,c,j] * W[d,j]  (DVE outer-product+reduce) ---
        PROD = sb.tile([128, NCH * D_OUT * D_IN], F32)
        xt_v = XT[:].rearrange("p (c j) -> p c j", j=D_IN)          # [128,64,4]
        xt_b = xt_v.unsqueeze(2).broadcast_to([128, NCH, D_OUT, D_IN])
        w_v = Wb[:, 0:12].rearrange("p (d j) -> p d j", j=D_IN)     # [128,3,4]
        w_b = w_v.unsqueeze(1).broadcast_to([128, NCH, D_OUT, D_IN])
        prod_v = PROD[:].rearrange("p (c d j) -> p c d j", d=D_OUT, j=D_IN)
        nc.vector.tensor_mul(prod_v, xt_b, w_b)
        Z = sb.tile([128, CD], F32)
        nc.vector.tensor_reduce(
            Z[:].rearrange("p (c d) -> p c d", d=D_OUT), prod_v,
            axis=mybir.AxisListType.X, op=mybir.AluOpType.add)

        # ZA = alpha * Z
        ZA = sb.tile([128, CD], F32)
        nc.vector.tensor_scalar_mul(ZA[:], Z[:], acol[:, 0:1])

        # u = exp(-alpha/2 * Z^2)  -> V[:, 0:CD]
        ZSQ = sb.tile([128, CD], F32)
        nc.scalar.activation(ZSQ[:], Z[:], mybir.ActivationFunctionType.Square)
        # V layout: col = c*KD + k*D_OUT + d  (chunk-major so each chunk's
        # matmul lhsT slice is a single contiguous free dim)
        V = sb.tile([128, NCH * KD], F32)
        v_ckd = V[:].rearrange("p (c k d) -> p c k d", k=NK, d=D_OUT)
        za_cd = ZA[:].rearrange("p (c d) -> p c d", d=D_OUT)
        # u lands in its own tile (ACT), then DVE copies it into V's k=0
        # slice so all V writers are DVE (matmul wait-count limit).
        U = sb.tile([128, CD], F32)
        nc.scalar.activation(U[:], ZSQ[:],
                             mybir.ActivationFunctionType.Exp, scale=nah[:, 0:1])
        u_cd = U[:].rearrange("p (c d) -> p c d", d=D_OUT)
        nc.vector.tensor_copy(v_ckd[:, :, 0, :], u_cd)

        # V_k = V_{k-1} * ZA   (u * (a z)^k)
        nc.vector.tensor_mul(v_ckd[:, :, 1, :], u_cd, za_cd)
        for k in range(2, NK):
            nc.vector.tensor_mul(v_ckd[:, :, k, :], v_ckd[:, :, k - 1, :], za_cd)

        # --- moments: psum[(k,d), (num,den)] += V_chunk^T @ [Y_c, 1] ---
        ps_m = ps.tile([KD, 2], F32)
        for c in range(NCH):
            lhsT = V[:, c * KD : (c + 1) * KD]   # [128, 51] contiguous
            rhs = YO[:, 2 * c : 2 * c + 2]       # [128, 2]
            nc.tensor.matmul(ps_m[:], lhsT, rhs,
                             start=(c == 0), stop=(c == NCH - 1))

        # scale by 1/k!
        msb = sb.tile([KD, 2], F32)
        nc.vector.tensor_mul(msb[:], ps_m[:], ifac)

        # transpose each [51,1] column -> [1,51], then broadcast to 128 partitions
        ps_tA = ps.tile([1, KD], F32)
        nc.tensor.transpose(ps_tA[:], msb[:, 0:1], ident[:])
        ps_tC = ps.tile([1, KD], F32)
        nc.tensor.transpose(ps_tC[:], msb[:, 1:2], ident[:])
        mtA = sb.tile([1, KD], F32)
        nc.vector.tensor_copy(mtA[:], ps_tA[:])
        mtC = sb.tile([1, KD], F32)
        nc.vector.tensor_copy(mtC[:], ps_tC[:])
        ps_A = ps.tile([128, KD], F32)
        nc.tensor.matmul(ps_A[:], ones_row[:], mtA[:], start=True, stop=True)
        ps_C = ps.tile([128, KD], F32)
        nc.tensor.matmul(ps_C[:], ones_row[:], mtC[:], start=True, stop=True)

        # --- query side (on gpsimd/Pool where possible, parallel with DVE) ---
        PRODQ = sb.tile([128, QC * D_OUT * D_IN], F32)
        xq_v = XQ[:].rearrange("p (c j) -> p c j", j=D_IN)
        xq_b = xq_v.unsqueeze(2).broadcast_to([128, QC, D_OUT, D_IN])
        wq_b = w_v.unsqueeze(1).broadcast_to([128, QC, D_OUT, D_IN])
        prodq_v = PRODQ[:].rearrange("p (c d j) -> p c d j", d=D_OUT, j=D_IN)
        nc.gpsimd.tensor_mul(prodq_v, xq_b, wq_b)
        XWQ = sb.tile([128, QCD], F32)
        nc.vector.tensor_reduce(
            XWQ[:].rearrange("p (c d) -> p c d", d=D_OUT), prodq_v,
            axis=mybir.AxisListType.X, op=mybir.AluOpType.add)

        Q = sb.tile([128, NK * QCD], F32)
        nc.gpsimd.memset(Q[:, 0:QCD], 1.0)
        for k in range(1, NK):
            nc.gpsimd.tensor_mul(Q[:, k * QCD : (k + 1) * QCD],
                                 Q[:, (k - 1) * QCD : k * QCD], XWQ[:])

        # num/den = sum_k coeff[k,d] * Q[:, k, c, d]
        q_v = Q[:].rearrange("p (k c d) -> p k c d", c=QC, d=D_OUT)
        a_b = ps_A[:].rearrange("p (k d) -> p k d", d=D_OUT) \
            .unsqueeze(2).broadcast_to([128, NK, QC, D_OUT])
        c_b = ps_C[:].rearrange("p (k d) -> p k d", d=D_OUT) \
            .unsqueeze(2).broadcast_to([128, NK, QC, D_OUT])
        TTN = sb.tile([128, NK * QCD], F32)
        ttn_v = TTN[:].rearrange("p (k c d) -> p k c d", c=QC, d=D_OUT)
        nc.vector.tensor_mul(ttn_v, q_v, a_b)
        TTD = sb.tile([128, NK * QCD], F32)
        ttd_v = TTD[:].rearrange("p (k c d) -> p k c d", c=QC, d=D_OUT)
        nc.vector.tensor_mul(ttd_v, q_v, c_b)

        NUMQ = sb.tile([128, QCD], F32)
        nc.vector.tensor_reduce(
            NUMQ[:].rearrange("p (c d) -> p c d", d=D_OUT),
            ttn_v.transpose([0, 2, 3, 1]),
            axis=mybir.AxisListType.X, op=mybir.AluOpType.add)
        DENQ = sb.tile([128, QCD], F32)
        nc.vector.tensor_reduce(
            DENQ[:].rearrange("p (c d) -> p c d", d=D_OUT),
            ttd_v.transpose([0, 2, 3, 1]),
            axis=mybir.AxisListType.X, op=mybir.AluOpType.add)

        RCP = sb.tile([128, QCD], F32)
        nc.vector.reciprocal(RCP[:], DENQ[:])
        OUTV = sb.tile([128, QCD], F32)
        nc.vector.tensor_mul(OUTV[:], NUMQ[:], RCP[:])

        nc.sync.dma_start(o_out[:, :].rearrange("(p c) d -> p (c d)", p=128),
                          OUTV[:])
    return nc


_NC_CACHE = None


def _get_nc():
    global _NC_CACHE
    if _NC_CACHE is None:
        nc = bacc.Bacc(
            "TRN2",
            target_bir_lowering=False,
            debug=False,
            enable_asserts=True,
            num_devices=N_CORES,
        )
        _emit(nc)
        nc.finalize()
        _NC_CACHE = nc
    return _NC_CACHE


def _const_inputs():
    tbl = np.zeros([KD, KD + 2], np.float32)
    for k in range(NK):
        tbl[k * D_OUT : (k + 1) * D_OUT, 0:2] = 1.0 / math.factorial(k)
    tbl[:, 2 : KD + 2] = np.eye(KD, dtype=np.float32)
    return tbl


def _run(x, train_X, Y, W, h, **spmd_kwargs):
    x = np.ascontiguousarray(np.asarray(x, np.float32))
    train_X = np.ascontiguousarray(np.asarray(train_X, np.float32))
    Y = np.ascontiguousarray(np.asarray(Y, np.float32))
    W = np.ascontiguousarray(np.asarray(W, np.float32))
    whp = np.concatenate(
        [W.reshape(-1), np.asarray(h, np.float32).reshape(-1)]).reshape(1, -1)
    tbl = _const_inputs()

    nc = _get_nc()
    in_maps = []
    for i in range(N_CORES):
        in_maps.append({
            "xq": x[i * B_LOC : (i + 1) * B_LOC],
            "train_x": train_X,
            "yv": Y,
            "whp": whp,
            "tbl": tbl,
        })
    return run_bass_kernel_spmd(nc, in_maps, list(range(N_CORES)), **spmd_kwargs)


def kernel(x, train_X, Y, W, h):
    res = _run(x, train_X, Y, W, h)
    out = np.concatenate([res.results[i]["out"] for i in range(N_CORES)], axis=0)
    return out.astype(np.float32)


# revision 12
# speedup vs baseline: 1.2532x; 1.2532x over previous
"""Trainium2 Bass kernel for Nadaraya-Watson kernel regression (retrieval_knn).

Reference computation (per output dim d, independently):
    z_d = train_X @ W[d]          [N]
    x_d = x @ W[d]                [B]
    k[n,b] = exp(-alpha/2 (z_n - x_b)^2),  alpha = 1/h^2
    out[b,d] = sum_n Y_n k[n,b] / sum_n k[n,b]

Instead of materializing the [N, B] kernel matrix (~100M exps), use the
factorization exp(-a/2(z-x)^2) = e^{-a z^2/2} e^{-a x^2/2} e^{a z x} and a
truncated Taylor expansion of e^{a z x} = sum_k (a z)^k x^k / k!.  The
e^{-a x^2/2} factor cancels in the num/den ratio, so:

    num[b,d] = sum_k A[k,d] x_d[b]^k,  A[k,d] = (1/k!) sum_n Y_n u[n,d] (a z)^k
    den[b,d] = sum_k C[k,d] x_d[b]^k,  C[k,d] = (1/k!) sum_n     u[n,d] (a z)^k
    u[n,d]   = exp(-a z_{n,d}^2 / 2)

with K=12 this matches the fp32 reference to ~1e-4 (validated numerically;
max |a z x| ~ 6.6 over the data distribution).

Sharding: batch B=4096 split across 8 cores (512 queries each); train side
replicated.  Each core computes the full train-side moments redundantly
(cheap) and evaluates its own queries.  No collectives.

Engine plan: train-side reduction over n is done as a DVE c-reduction
(numerator, Y-weighted) plus a GpSimd pairwise add tree (denominator),
followed by a single PE matmul against a ones column for the 128-partition
reduction.  The Vandermonde build is split into even (DVE) / odd (GpSimd)
power chains.  The query side runs on GpSimd in parallel.
"""

import math
from contextlib import ExitStack

import numpy as np

import concourse.bass as bass
import concourse.tile as tile
from concourse import bacc, mybir
from concourse.bass_utils import run_bass_kernel_spmd

F32 = mybir.dt.float32

N_TRAIN = 8192
B = 4096
D_IN = 4
D_OUT = 3
N_CORES = 8
B_LOC = B // N_CORES          # 512 queries per core
NCH = N_TRAIN // 128          # 64 train chunks (partition dim)
CD = NCH * D_OUT              # 192  (c, d) columns
K_DEG = 12                    # Taylor degree
NK = K_DEG + 1                # 13 terms
KD = NK * D_OUT               # 39  (k, d) moment columns
KD2 = 2 * KD                  # 78  (num | den)
QC = B_LOC // 128             # 4 query chunks
QCD = QC * D_OUT              # 12


def _emit(nc: bass.Bass):
    x_in = nc.declare_dram_parameter("xq", [B_LOC, D_IN], F32, isOutput=False)
    tx_in = nc.declare_dram_parameter("train_x", [N_TRAIN, D_IN], F32, isOutput=False)
    y_in = nc.declare_dram_parameter("yv", [N_TRAIN], F32, isOutput=False)
    wh_in = nc.declare_dram_parameter("whp", [1, D_OUT * D_IN + 1], F32, isOutput=False)
    tbl_in = nc.declare_dram_parameter("tbl", [KD2, KD2 + 1], F32, isOutput=False)
    o_out = nc.declare_dram_parameter("out", [B_LOC, D_OUT], F32, isOutput=True)

    with tile.TileContext(nc) as tc, ExitStack() as ctx:
        sb = ctx.enter_context(tc.tile_pool(name="sb", bufs=1))
        ps = ctx.enter_context(tc.tile_pool(name="ps", bufs=1, space="PSUM"))

        # --- ACT table preload (overlaps with DMAs) ---
        warm = sb.tile([1, 1], F32)
        nc.vector.memset(warm[:], 0.0)
        nc.scalar.activation(warm[:], warm[:], mybir.ActivationFunctionType.Square)
        nc.scalar.activation(warm[:], warm[:], mybir.ActivationFunctionType.Exp)

        # --- input DMAs (all on the SP/HWDGE queue; generation pipelines) ---
        # W flat + h in one packed [1, 13] row
        wh_row = sb.tile([1, D_OUT * D_IN + 1], F32)
        nc.sync.dma_start(wh_row[:], wh_in[:, :])

        # train_X rows n = p*64 + c  ->  XT[p, c*4 + j]   (1KB contig/partition)
        XT = sb.tile([128, NCH * D_IN], F32)
        nc.sync.dma_start(XT[:], tx_in[:, :].rearrange("(p c) d -> p (c d)", p=128))

        # queries: rows b = p*4 + c -> XQ[p, c*4 + j]
        XQ = sb.tile([128, QC * D_IN], F32)
        nc.sync.dma_start(XQ[:], x_in[:, :].rearrange("(p c) d -> p (c d)", p=128))

        # Y with the same n = p*64 + c mapping
        YT = sb.tile([128, NCH], F32)
        nc.sync.dma_start(YT[:], y_in[:].rearrange("(p c) -> p c", p=128))

        # const tables: col 0 = 1/k! (78 rows: num block | den block),
        # cols 1..79 = identity(78)
        tblT = sb.tile([KD2, KD2 + 1], F32)
        nc.sync.dma_start(tblT[:], tbl_in[:, :])
        ident = sb.tile([KD2, KD2], F32)
        nc.vector.tensor_copy(ident[:], tblT[:, 1 : KD2 + 1])

        # --- broadcast W,h across partitions via ones-matmul ---
        ones_row = sb.tile([1, 128], F32)
        nc.vector.memset(ones_row[:], 1.0)
        whr2 = sb.tile([1, D_OUT * D_IN + 1], F32)
        nc.vector.tensor_copy(whr2[:], wh_row[:])
        ps_wh = ps.tile([128, D_OUT * D_IN + 1], F32)
        nc.tensor.matmul(ps_wh[:], ones_row[:], whr2[:], start=True, stop=True)
        Wb = sb.tile([128, D_OUT * D_IN + 1], F32)
        nc.scalar.copy(Wb[:], ps_wh[:])

        # --- alpha = 1/h^2 per-partition columns ---
        hcol = Wb[:, 12:13]
        h2 = sb.tile([128, 1], F32)
        nc.vector.tensor_mul(h2[:], hcol, hcol)
        acol = sb.tile([128, 1], F32)
        nc.vector.reciprocal(acol[:], h2[:])
        nah = sb.tile([128, 1], F32)      # -alpha/2
        nc.vector.tensor_scalar_mul(nah[:], acol[:], -0.5)

        # --- Z[p, c*3+d] = sum_j XT[p,c,j] * W[d,j]  (DVE outer-product+reduce) ---
        PROD = sb.tile([128, NCH * D_OUT * D_IN], F32)
        xt_v = XT[:].rearrange("p (c j) -> p c j", j=D_IN)          # [128,64,4]
        xt_b = xt_v.unsqueeze(2).broadcast_to([128, NCH, D_OUT, D_IN])
        w_v = Wb[:, 0:12].rearrange("p (d j) -> p d j", j=D_IN)     # [128,3,4]
        w_b = w_v.unsqueeze(1).broadcast_to([128, NCH, D_OUT, D_IN])
        prod_v = PROD[:].rearrange("p (c d j) -> p c d j", d=D_OUT, j=D_IN)
        nc.vector.tensor_mul(prod_v, xt_b, w_b)
        Z = sb.tile([128, CD], F32)
        nc.vector.tensor_reduce(
            Z[:].rearrange("p (c d) -> p c d", d=D_OUT), prod_v,
            axis=mybir.AxisListType.X, op=mybir.AluOpType.add)

        # ZA = alpha * Z ; ZA2 = ZA^2
        ZA = sb.tile([128, CD], F32)
        nc.vector.tensor_scalar_mul(ZA[:], Z[:], acol[:, 0:1])
        ZA2 = sb.tile([128, CD], F32)
        nc.vector.tensor_mul(ZA2[:], ZA[:], ZA[:])

        # u = exp(-alpha/2 * Z^2)  (ACT)
        ZSQ = sb.tile([128, CD], F32)
        nc.scalar.activation(ZSQ[:], Z[:], mybir.ActivationFunctionType.Square)
        U = sb.tile([128, CD], F32)
        nc.scalar.activation(U[:], ZSQ[:],
                             mybir.ActivationFunctionType.Exp, scale=nah[:, 0:1])
        u_cd = U[:].rearrange("p (c d) -> p c d", d=D_OUT)

        # V layout: col = c*KD + k*D_OUT + d.  Two power chains:
        #   even k on DVE:  V0 = u, V_{k+2} = V_k * ZA2
        #   odd  k on Pool: V1 = u*ZA, V_{k+2} = V_k * ZA2
        V = sb.tile([128, NCH * KD], F32)
        v_ckd = V[:].rearrange("p (c k d) -> p c k d", k=NK, d=D_OUT)
        za_cd = ZA[:].rearrange("p (c d) -> p c d", d=D_OUT)
        za2_cd = ZA2[:].rearrange("p (c d) -> p c d", d=D_OUT)
        nc.vector.tensor_copy(v_ckd[:, :, 0, :], u_cd)
        nc.gpsimd.tensor_mul(v_ckd[:, :, 1, :], u_cd, za_cd)
        for k in range(2, NK):
            eng = nc.vector if k % 2 == 0 else nc.gpsimd
            eng.tensor_mul(v_ckd[:, :, k, :], v_ckd[:, :, k - 2, :], za2_cd)

        # --- moments ---
        # PART[:, 0:39]  = sum_c Y*V   (DVE: weighted mul then c-reduce)
        # PART[:, 39:78] = sum_c   V   (GpSimd: pairwise add tree)
        PART = sb.tile([128, KD2], F32)

        VY = sb.tile([128, NCH * KD], F32)
        y_b = YT[:].unsqueeze(2).unsqueeze(3).broadcast_to([128, NCH, NK, D_OUT])
        nc.vector.tensor_mul(
            VY[:].rearrange("p (c k d) -> p c k d", k=NK, d=D_OUT), v_ckd, y_b)
        vy_t = VY[:].rearrange("p (c k d) -> p c k d", k=NK, d=D_OUT) \
            .transpose([0, 2, 3, 1])                       # [128, 13, 3, 64]
        nc.vector.tensor_reduce(
            PART[:, 0:KD].rearrange("p (k d) -> p k d", d=D_OUT), vy_t,
            axis=mybir.AxisListType.X, op=mybir.AluOpType.add)

        # den tree on GpSimd: 64 -> 32 -> 16 -> 8 -> 4 -> 2 -> 1 chunks
        DTS = sb.tile([128, NCH * KD], F32)
        src = V[:]
        width = NCH
        off = 0
        while width > 2:
            half = width // 2
            dst = DTS[:, off : off + half * KD]
            nc.gpsimd.tensor_add(
                dst.rearrange("p (c e) -> p c e", e=KD),
                src.rearrange("p (two c e) -> p c two e", two=2, e=KD)[:, :, 0],
                src.rearrange("p (two c e) -> p c two e", two=2, e=KD)[:, :, 1])
            src = dst
            off += half * KD
            width = half
        nc.gpsimd.tensor_add(PART[:, KD:KD2], src[:, 0:KD], src[:, KD : 2 * KD])

        # --- single partition-reduction matmul: psum[j, 0] = sum_p PART[p, j] ---
        ones_col = sb.tile([128, 1], F32)
        nc.vector.memset(ones_col[:], 1.0)
        ps_m = ps.tile([KD2, 1], F32)
        nc.tensor.matmul(ps_m[:], PART[:], ones_col[:], start=True, stop=True)

        # scale by 1/k!
        msb = sb.tile([KD2, 1], F32)
        nc.vector.tensor_mul(msb[:], ps_m[:], tblT[:, 0:1])

        # transpose [78,1] -> [1,78], broadcast to 128 partitions
        ps_t = ps.tile([1, KD2], F32)
        nc.tensor.transpose(ps_t[:], msb[:], ident[:])
        mt = sb.tile([1, KD2], F32)
        nc.vector.tensor_copy(mt[:], ps_t[:])
        ps_AB = ps.tile([128, KD2], F32)
        nc.tensor.matmul(ps_AB[:], ones_row[:], mt[:], start=True, stop=True)

        # --- query side (GpSimd where possible, parallel with train side) ---
        PRODQ = sb.tile([128, QC * D_OUT * D_IN], F32)
        xq_v = XQ[:].rearrange("p (c j) -> p c j", j=D_IN)
        xq_b = xq_v.unsqueeze(2).broadcast_to([128, QC, D_OUT, D_IN])
        wq_b = w_v.unsqueeze(1).broadcast_to([128, QC, D_OUT, D_IN])
        prodq_v = PRODQ[:].rearrange("p (c d j) -> p c d j", d=D_OUT, j=D_IN)
        nc.gpsimd.tensor_mul(prodq_v, xq_b, wq_b)
        XWQ = sb.tile([128, QCD], F32)
        nc.vector.tensor_reduce(
            XWQ[:].rearrange("p (c d) -> p c d", d=D_OUT), prodq_v,
            axis=mybir.AxisListType.X, op=mybir.AluOpType.add)

        # Q layout: col = c*NK*D_OUT + k*D_OUT + d; even/odd chains via XW^2
        Q = sb.tile([128, QC * KD], F32)
        q_ckd = Q[:].rearrange("p (c k d) -> p c k d", k=NK, d=D_OUT)
        xw_cd = XWQ[:].rearrange("p (c d) -> p c d", d=D_OUT)
        nc.gpsimd.memset(q_ckd[:, :, 0, :], 1.0)
        nc.gpsimd.tensor_copy(q_ckd[:, :, 1, :], xw_cd)
        XW2 = sb.tile([128, QCD], F32)
        nc.gpsimd.tensor_mul(XW2[:], XWQ[:], XWQ[:])
        xw2_cd = XW2[:].rearrange("p (c d) -> p c d", d=D_OUT)
        for k in range(2, NK):
            nc.gpsimd.tensor_mul(q_ckd[:, :, k, :], q_ckd[:, :, k - 2, :], xw2_cd)

        # num/den = sum_k coeff[k,d] * Q[:, c, k, d]
        a_b = ps_AB[:, 0:KD].rearrange("p (k d) -> p k d", d=D_OUT) \
            .unsqueeze(1).broadcast_to([128, QC, NK, D_OUT])
        c_b = ps_AB[:, KD:KD2].rearrange("p (k d) -> p k d", d=D_OUT) \
            .unsqueeze(1).broadcast_to([128, QC, NK, D_OUT])
        TTN = sb.tile([128, QC * KD], F32)
        ttn_v = TTN[:].rearrange("p (c k d) -> p c k d", k=NK, d=D_OUT)
        nc.vector.tensor_mul(ttn_v, q_ckd, a_b)
        TTD = sb.tile([128, QC * KD], F32)
        ttd_v = TTD[:].rearrange("p (c k d) -> p c k d", k=NK, d=D_OUT)
        nc.vector.tensor_mul(ttd_v, q_ckd, c_b)

        NUMQ = sb.tile([128, QCD], F32)
        nc.vector.tensor_reduce(
            NUMQ[:].rearrange("p (c d) -> p c d", d=D_OUT),
            ttn_v.transpose([0, 1, 3, 2]),
            axis=mybir.AxisListType.X, op=mybir.AluOpType.add)
        DENQ = sb.tile([128, QCD], F32)
        nc.vector.tensor_reduce(
            DENQ[:].rearrange("p (c d) -> p c d", d=D_OUT),
            ttd_v.transpose([0, 1, 3, 2]),
            axis=mybir.AxisListType.X, op=mybir.AluOpType.add)

        RCP = sb.tile([128, QCD], F32)
        nc.vector.reciprocal(RCP[:], DENQ[:])
        OUTV = sb.tile([128, QCD], F32)
        nc.vector.tensor_mul(OUTV[:], NUMQ[:], RCP[:])

        nc.sync.dma_start(o_out[:, :].rearrange("(p c) d -> p (c d)", p=128),
                          OUTV[:])
    return nc


_NC_CACHE = None


def _get_nc():
    global _NC_CACHE
    if _NC_CACHE is None:
        nc = bacc.Bacc(
            "TRN2",
            target_bir_lowering=False,
            debug=False,
            enable_asserts=True,
            num_devices=N_CORES,
        )
        _emit(nc)
        nc.finalize()
        _NC_CACHE = nc
    return _NC_CACHE


def _const_inputs():
    tbl = np.zeros([KD2, KD2 + 1], np.float32)
    for k in range(NK):
        tbl[k * D_OUT : (k + 1) * D_OUT, 0] = 1.0 / math.factorial(k)
        tbl[KD + k * D_OUT : KD + (k + 1) * D_OUT, 0] = 1.0 / math.factorial(k)
    tbl[:, 1 : KD2 + 1] = np.eye(KD2, dtype=np.float32)
    return tbl


def _run(x, train_X, Y, W, h, **spmd_kwargs):
    x = np.ascontiguousarray(np.asarray(x, np.float32))
    train_X = np.ascontiguousarray(np.asarray(train_X, np.float32))
    Y = np.ascontiguousarray(np.asarray(Y, np.float32))
    W = np.ascontiguousarray(np.asarray(W, np.float32))
    whp = np.concatenate(
        [W.reshape(-1), np.asarray(h, np.float32).reshape(-1)]).reshape(1, -1)
    tbl = _const_inputs()

    nc = _get_nc()
    in_maps = []
    for i in range(N_CORES):
        in_maps.append({
            "xq": x[i * B_LOC : (i + 1) * B_LOC],
            "train_x": train_X,
            "yv": Y,
            "whp": whp,
            "tbl": tbl,
        })
    return run_bass_kernel_spmd(nc, in_maps, list(range(N_CORES)), **spmd_kwargs)


def kernel(x, train_X, Y, W, h):
    res = _run(x, train_X, Y, W, h)
    out = np.concatenate([res.results[i]["out"] for i in range(N_CORES)], axis=0)
    return out.astype(np.float32)


# revision 14
# speedup vs baseline: 1.3150x; 1.0493x over previous
"""Trainium2 Bass kernel for Nadaraya-Watson kernel regression (retrieval_knn).

Reference computation (per output dim d, independently):
    z_d = train_X @ W[d]          [N]
    x_d = x @ W[d]                [B]
    k[n,b] = exp(-alpha/2 (z_n - x_b)^2),  alpha = 1/h^2
    out[b,d] = sum_n Y_n k[n,b] / sum_n k[n,b]

Instead of materializing the [N, B] kernel matrix (~100M exps), use the
factorization exp(-a/2(z-x)^2) = e^{-a z^2/2} e^{-a x^2/2} e^{a z x} and a
truncated Taylor expansion of e^{a z x} = sum_k (a z)^k x^k / k!.  The
e^{-a x^2/2} factor cancels in the num/den ratio, so:

    num[b,d] = sum_k A[k,d] x_d[b]^k,  A[k,d] = (1/k!) sum_n Y_n u[n,d] (a z)^k
    den[b,d] = sum_k C[k,d] x_d[b]^k,  C[k,d] = (1/k!) sum_n     u[n,d] (a z)^k
    u[n,d]   = exp(-a z_{n,d}^2 / 2)

with K=12 this matches the fp32 reference to ~1e-4 (validated numerically;
max |a z x| ~ 6.6 over the data distribution).

Sharding: batch B=4096 split across 8 cores (512 queries each); train side
replicated.  Each core computes the full train-side moments redundantly
(cheap) and evaluates its own queries.  No collectives.

Train side layout is (k, d, c) with c (the 64 row-chunks) innermost, so the
big Y-weighting multiply and the two chunk reductions are contiguous DVE
ops.  The 128-partition reduction is a single PE matmul against a ones
column.  The query side runs on GpSimd in parallel.  The Tile end-of-kernel
semaphore-wait storm is replaced by a lean drain (a WAR sentinel on the
output tile guarantees DMA completion before the final barrier).
"""

import math
from contextlib import ExitStack

import numpy as np

import concourse.bass as bass
import concourse.tile as tile
from concourse import bacc, mybir
from concourse.bass_utils import run_bass_kernel_spmd
from concourse.vector_clock import ScopedClock

F32 = mybir.dt.float32

N_TRAIN = 8192
B = 4096
D_IN = 4
D_OUT = 3
N_CORES = 8
B_LOC = B // N_CORES          # 512 queries per core
NCH = N_TRAIN // 128          # 64 train chunks (partition dim)
CD = NCH * D_OUT              # 192  (d, c) columns
K_DEG = 12                    # Taylor degree
NK = K_DEG + 1                # 13 terms
KD = NK * D_OUT               # 39  (k, d) moment columns
KD2 = 2 * KD                  # 78  (num | den)
QC = B_LOC // 128             # 4 query chunks
QCD = QC * D_OUT              # 12


def _lean_drain_and_barrier(self, tick_clock, wait_clock):
    """Replacement for TileContext._drain_and_barrier without the per-sem
    wait storm.  All compute semaphores are at final values once every
    engine reaches the barrier (engine program order), and DMA completion
    is guaranteed by WAR sentinels inside the program, so the final drain
    does not need to wait on each semaphore individually."""
    self.nc.sync.drain()
    self.nc.all_engine_barrier()
    popped = self.nc._tile_sem_poison_stack.pop()
    assert popped is self._sem_poison
    self.nc.clear_and_free_semaphores(list(self.sems.allocated().values()))
    self.nc.all_engine_barrier()


def _emit(nc: bass.Bass):
    x_in = nc.declare_dram_parameter("xq", [B_LOC, D_IN], F32, isOutput=False)
    tx_in = nc.declare_dram_parameter("train_x", [N_TRAIN, D_IN], F32, isOutput=False)
    y_in = nc.declare_dram_parameter("yv", [N_TRAIN], F32, isOutput=False)
    wh_in = nc.declare_dram_parameter("whp", [1, D_OUT * D_IN + 1], F32, isOutput=False)
    tbl_in = nc.declare_dram_parameter("tbl", [KD2, KD2 + 1], F32, isOutput=False)
    o_out = nc.declare_dram_parameter("out", [B_LOC, D_OUT], F32, isOutput=True)

    with tile.TileContext(nc) as tc, ExitStack() as ctx:
        sb = ctx.enter_context(tc.tile_pool(name="sb", bufs=1))
        ps = ctx.enter_context(tc.tile_pool(name="ps", bufs=1, space="PSUM"))

        # --- ACT table preload (overlaps with DMAs) ---
        warm = sb.tile([1, 1], F32)
        nc.gpsimd.memset(warm[:], 0.0)
        nc.scalar.activation(warm[:], warm[:], mybir.ActivationFunctionType.Square)
        nc.scalar.activation(warm[:], warm[:], mybir.ActivationFunctionType.Exp)

        # --- input DMAs, spread across DGE queues ---
        wh_row = sb.tile([1, D_OUT * D_IN + 1], F32)
        nc.sync.dma_start(wh_row[:], wh_in[:, :])

        # train_X rows n = p*64 + c  ->  XT[p, c*4 + j]   (1KB contig/partition)
        XT = sb.tile([128, NCH * D_IN], F32)
        nc.sync.dma_start(XT[:], tx_in[:, :].rearrange("(p c) d -> p (c d)", p=128))

        # Y with the same n = p*64 + c mapping
        YT = sb.tile([128, NCH], F32)
        nc.sync.dma_start(YT[:], y_in[:].rearrange("(p c) -> p c", p=128))

        # queries: rows b = p*4 + c -> XQ[p, c*4 + j]
        XQ = sb.tile([128, QC * D_IN], F32)
        nc.scalar.dma_start(XQ[:], x_in[:, :].rearrange("(p c) d -> p (c d)", p=128))

        # const tables: col 0 = 1/k! (78 rows: num | den), cols 1..79 = I(78)
        tblT = sb.tile([KD2, KD2 + 1], F32)
        nc.gpsimd.dma_start(tblT[:], tbl_in[:, :])

        # --- broadcast W,h across partitions via ones-matmul ---
        ones_row = sb.tile([1, 128], F32)
        nc.vector.memset(ones_row[:], 1.0)
        whr2 = sb.tile([1, D_OUT * D_IN + 1], F32)
        nc.vector.tensor_copy(whr2[:], wh_row[:])
        ps_wh = ps.tile([128, D_OUT * D_IN + 1], F32)
        nc.tensor.matmul(ps_wh[:], ones_row[:], whr2[:], start=True, stop=True)
        Wb = sb.tile([128, D_OUT * D_IN + 1], F32)
        nc.scalar.copy(Wb[:], ps_wh[:])

        # --- alpha = 1/h^2 per-partition columns ---
        hcol = Wb[:, 12:13]
        h2 = sb.tile([128, 1], F32)
        nc.vector.tensor_mul(h2[:], hcol, hcol)
        acol = sb.tile([128, 1], F32)
        nc.vector.reciprocal(acol[:], h2[:])
        nah = sb.tile([128, 1], F32)      # -alpha/2
        nc.vector.tensor_scalar_mul(nah[:], acol[:], -0.5)

        # --- Z[p, d*64+c] = sum_j XT[p,c,j] * W[d,j]  (DVE, (d,c) layout) ---
        PROD = sb.tile([128, D_OUT * NCH * D_IN], F32)
        xt_v = XT[:].rearrange("p (c j) -> p c j", j=D_IN)          # [128,64,4]
        xt_b = xt_v.unsqueeze(1).broadcast_to([128, D_OUT, NCH, D_IN])
        w_v = Wb[:, 0:12].rearrange("p (d j) -> p d j", j=D_IN)     # [128,3,4]
        w_b = w_v.unsqueeze(2).broadcast_to([128, D_OUT, NCH, D_IN])
        prod_v = PROD[:].rearrange("p (d c j) -> p d c j", c=NCH, j=D_IN)
        nc.vector.tensor_mul(prod_v, xt_b, w_b)
        Z = sb.tile([128, CD], F32)
        nc.vector.tensor_reduce(
            Z[:].rearrange("p (d c) -> p d c", c=NCH), prod_v,
            axis=mybir.AxisListType.X, op=mybir.AluOpType.add)

        # ZA = alpha * Z ; ZA2 = ZA^2
        ZA = sb.tile([128, CD], F32)
        nc.vector.tensor_scalar_mul(ZA[:], Z[:], acol[:, 0:1])
        ZA2 = sb.tile([128, CD], F32)
        nc.vector.tensor_mul(ZA2[:], ZA[:], ZA[:])

        # u = exp(-alpha/2 * Z^2)  (ACT)
        ZSQ = sb.tile([128, CD], F32)
        nc.scalar.activation(ZSQ[:], Z[:], mybir.ActivationFunctionType.Square)

        # V layout: col = k*CD + d*NCH + c; V_k contiguous [128, 192] blocks.
        # V0 = u (ACT writes it directly), V1 = u*ZA, V_k = V_{k-2}*ZA2 (DVE)
        V = sb.tile([128, NK * CD], F32)
        nc.scalar.activation(V[:, 0:CD], ZSQ[:],
                             mybir.ActivationFunctionType.Exp, scale=nah[:, 0:1])
        nc.vector.tensor_mul(V[:, CD : 2 * CD], V[:, 0:CD], ZA[:])
        for k in range(2, NK):
            nc.vector.tensor_mul(V[:, k * CD : (k + 1) * CD],
                                 V[:, (k - 2) * CD : (k - 1) * CD], ZA2[:])

        # --- moments ---
        # PART[:, 0:39]  = sum_c Y*V   (DVE: weighted mul then c-reduce)
        # PART[:, 39:78] = sum_c   V   (DVE c-reduce)
        PART = sb.tile([128, KD2], F32)
        v_kdc = V[:].rearrange("p (e c) -> p e c", c=NCH)          # e = (k,d)
        VY = sb.tile([128, NK * CD], F32)
        y_b = YT[:].unsqueeze(1).broadcast_to([128, KD, NCH])
        nc.vector.tensor_mul(
            VY[:].rearrange("p (e c) -> p e c", c=NCH), v_kdc, y_b)
        nc.vector.tensor_reduce(
            PART[:, 0:KD], VY[:].rearrange("p (e c) -> p e c", c=NCH),
            axis=mybir.AxisListType.X, op=mybir.AluOpType.add)
        nc.vector.tensor_reduce(
            PART[:, KD:KD2], v_kdc,
            axis=mybir.AxisListType.X, op=mybir.AluOpType.add)

        # --- single partition-reduction matmul: psum[j, 0] = sum_p PART[p, j] ---
        ones_col = sb.tile([128, 1], F32)
        nc.vector.memset(ones_col[:], 1.0)
        ps_m = ps.tile([KD2, 1], F32)
        nc.tensor.matmul(ps_m[:], PART[:], ones_col[:], start=True, stop=True)

        # scale by 1/k!
        msb = sb.tile([KD2, 1], F32)
        nc.vector.tensor_mul(msb[:], ps_m[:], tblT[:, 0:1])

        # transpose [78,1] -> [1,78], broadcast to 128 partitions
        ps_t = ps.tile([1, KD2], F32)
        nc.tensor.transpose(ps_t[:], msb[:], tblT[:, 1 : KD2 + 1])
        mt = sb.tile([1, KD2], F32)
        nc.vector.tensor_copy(mt[:], ps_t[:])
        ps_AB = ps.tile([128, KD2], F32)
        nc.tensor.matmul(ps_AB[:], ones_row[:], mt[:], start=True, stop=True)

        # --- query side (GpSimd, parallel with train side) ---
        PRODQ = sb.tile([128, QC * D_OUT * D_IN], F32)
        xq_v = XQ[:].rearrange("p (c j) -> p c j", j=D_IN)
        xq_b = xq_v.unsqueeze(2).broadcast_to([128, QC, D_OUT, D_IN])
        wq_b = w_v.unsqueeze(1).broadcast_to([128, QC, D_OUT, D_IN])
        prodq_v = PRODQ[:].rearrange("p (c d j) -> p c d j", d=D_OUT, j=D_IN)
        nc.gpsimd.tensor_mul(prodq_v, xq_b, wq_b)
        XWQ = sb.tile([128, QCD], F32)
        nc.vector.tensor_reduce(
            XWQ[:].rearrange("p (c d) -> p c d", d=D_OUT), prodq_v,
            axis=mybir.AxisListType.X, op=mybir.AluOpType.add)

        # Q layout: col = c*NK*D_OUT + k*D_OUT + d; even/odd chains via XW^2
        Q = sb.tile([128, QC * KD], F32)
        q_ckd = Q[:].rearrange("p (c k d) -> p c k d", k=NK, d=D_OUT)
        xw_cd = XWQ[:].rearrange("p (c d) -> p c d", d=D_OUT)
        nc.gpsimd.memset(q_ckd[:, :, 0, :], 1.0)
        nc.gpsimd.tensor_copy(q_ckd[:, :, 1, :], xw_cd)
        XW2 = sb.tile([128, QCD], F32)
        nc.gpsimd.tensor_mul(XW2[:], XWQ[:], XWQ[:])
        xw2_cd = XW2[:].rearrange("p (c d) -> p c d", d=D_OUT)
        for k in range(2, NK):
            nc.gpsimd.tensor_mul(q_ckd[:, :, k, :], q_ckd[:, :, k - 2, :], xw2_cd)

        # num/den = sum_k coeff[k,d] * Q[:, c, k, d]
        a_b = ps_AB[:, 0:KD].rearrange("p (k d) -> p k d", d=D_OUT) \
            .unsqueeze(1).broadcast_to([128, QC, NK, D_OUT])
        c_b = ps_AB[:, KD:KD2].rearrange("p (k d) -> p k d", d=D_OUT) \
            .unsqueeze(1).broadcast_to([128, QC, NK, D_OUT])
        TTN = sb.tile([128, QC * KD], F32)
        ttn_v = TTN[:].rearrange("p (c k d) -> p c k d", k=NK, d=D_OUT)
        nc.vector.tensor_mul(ttn_v, q_ckd, a_b)
        TTD = sb.tile([128, QC * KD], F32)
        ttd_v = TTD[:].rearrange("p (c k d) -> p c k d", k=NK, d=D_OUT)
        nc.vector.tensor_mul(ttd_v, q_ckd, c_b)

        NUMQ = sb.tile([128, QCD], F32)
        nc.vector.tensor_reduce(
            NUMQ[:].rearrange("p (c d) -> p c d", d=D_OUT),
            ttn_v.transpose([0, 1, 3, 2]),
            axis=mybir.AxisListType.X, op=mybir.AluOpType.add)
        DENQ = sb.tile([128, QCD], F32)
        nc.vector.tensor_reduce(
            DENQ[:].rearrange("p (c d) -> p c d", d=D_OUT),
            ttd_v.transpose([0, 1, 3, 2]),
            axis=mybir.AxisListType.X, op=mybir.AluOpType.add)

        RCP = sb.tile([128, QCD], F32)
        nc.vector.reciprocal(RCP[:], DENQ[:])
        OUTV = sb.tile([128, QCD], F32)
        nc.vector.tensor_mul(OUTV[:], NUMQ[:], RCP[:])

        nc.sync.dma_start(o_out[:, :].rearrange("(p c) d -> p (c d)", p=128),
                          OUTV[:])
        # WAR sentinel: overwriting OUTV forces a wait for the out-DMA's
        # completion, so the lean tail barrier needs no per-sem waits.
        nc.vector.memset(OUTV[0:1, 0:1], 0.0)
    return nc


_NC_CACHE = None


def _get_nc():
    global _NC_CACHE
    if _NC_CACHE is None:
        orig = tile.TileContext._drain_and_barrier
        tile.TileContext._drain_and_barrier = _lean_drain_and_barrier
        try:
            nc = bacc.Bacc(
                "TRN2",
                target_bir_lowering=False,
                debug=False,
                enable_asserts=True,
                num_devices=N_CORES,
            )
            _emit(nc)
            nc.finalize()
        finally:
            tile.TileContext._drain_and_barrier = orig
        _NC_CACHE = nc
    return _NC_CACHE


def _const_inputs():
    tbl = np.zeros([KD2, KD2 + 1], np.float32)
    for k in range(NK):
        tbl[k * D_OUT : (k + 1) * D_OUT, 0] = 1.0 / math.factorial(k)
        tbl[KD + k * D_OUT : KD + (k + 1) * D_OUT, 0] = 1.0 / math.factorial(k)
    tbl[:, 1 : KD2 + 1] = np.eye(KD2, dtype=np.float32)
    return tbl


def _run(x, train_X, Y, W, h, **spmd_kwargs):
    x = np.ascontiguousarray(np.asarray(x, np.float32))
    train_X = np.ascontiguousarray(np.asarray(train_X, np.float32))
    Y = np.ascontiguousarray(np.asarray(Y, np.float32))
    W = np.ascontiguousarray(np.asarray(W, np.float32))
    whp = np.concatenate(
        [W.reshape(-1), np.asarray(h, np.float32).reshape(-1)]).reshape(1, -1)
    tbl = _const_inputs()

    nc = _get_nc()
    in_maps = []
    for i in range(N_CORES):
        in_maps.append({
            "xq": x[i * B_LOC : (i + 1) * B_LOC],
            "train_x": train_X,
            "yv": Y,
            "whp": whp,
            "tbl": tbl,
        })
    return run_bass_kernel_spmd(nc, in_maps, list(range(N_CORES)), **spmd_kwargs)


def kernel(x, train_X, Y, W, h):
    res = _run(x, train_X, Y, W, h)
    out = np.concatenate([res.results[i]["out"] for i in range(N_CORES)], axis=0)
    return out.astype(np.float32)


# revision 16
# speedup vs baseline: 1.4099x; 1.0722x over previous
"""Trainium2 Bass kernel for Nadaraya-Watson kernel regression (retrieval_knn).

Reference computation (per output dim d, independently):
    z_d = train_X @ W[d]          [N]
    x_d = x @ W[d]                [B]
    k[n,b] = exp(-alpha/2 (z_n - x_b)^2),  alpha = 1/h^2
    out[b,d] = sum_n Y_n k[n,b] / sum_n k[n,b]

Instead of materializing the [N, B] kernel matrix (~100M exps), use the
factorization exp(-a/2(z-x)^2) = e^{-a z^2/2} e^{-a x^2/2} e^{a z x} and a
truncated Taylor expansion of e^{a z x} = sum_k (a z)^k x^k / k!.  The
e^{-a x^2/2} factor cancels in the num/den ratio, so:

    num[b,d] = sum_k A[k,d] x_d[b]^k,  A[k,d] = (1/k!) sum_n Y_n u[n,d] (a z)^k
    den[b,d] = sum_k C[k,d] x_d[b]^k,  C[k,d] = (1/k!) sum_n     u[n,d] (a z)^k
    u[n,d]   = exp(-a z_{n,d}^2 / 2)

with K=12 this matches the fp32 reference to ~1e-4 (validated numerically;
max |a z x| ~ 6.6 over the data distribution).

Sharding: batch B=4096 split across 8 cores (512 queries each); train side
replicated.  Each core computes the full train-side moments redundantly
(cheap) and evaluates its own queries.  No collectives.

Train side layout is (k, d, c) with c (the 64 row-chunks) innermost, so the
big Y-weighting multiply and the two chunk reductions are contiguous DVE
ops.  The 128-partition reduction is a single PE matmul against a ones
column.  The query side runs on GpSimd in parallel.  The Tile end-of-kernel
semaphore-wait storm is replaced by a lean drain (a WAR sentinel on the
output tile guarantees DMA completion before the final barrier).
"""

import math
from contextlib import ExitStack

import numpy as np

import concourse.bass as bass
import concourse.tile as tile
from concourse import bacc, mybir
from concourse.bass_utils import run_bass_kernel_spmd
from concourse.vector_clock import ScopedClock

F32 = mybir.dt.float32

N_TRAIN = 8192
B = 4096
D_IN = 4
D_OUT = 3
N_CORES = 8
B_LOC = B // N_CORES          # 512 queries per core
NCH = N_TRAIN // 128          # 64 train chunks (partition dim)
CD = NCH * D_OUT              # 192  (d, c) columns
K_DEG = 12                    # Taylor degree
NK = K_DEG + 1                # 13 terms
KD = NK * D_OUT               # 39  (k, d) moment columns
KD2 = 2 * KD                  # 78  (num | den)
QC = B_LOC // 128             # 4 query chunks
QCD = QC * D_OUT              # 12


def _lean_drain_and_barrier(self, tick_clock, wait_clock):
    """Replacement for TileContext._drain_and_barrier without the per-sem
    wait storm.  All compute semaphores are at final values once every
    engine reaches the barrier (engine program order), and DMA completion
    is guaranteed by WAR sentinels inside the program, so the final drain
    does not need to wait on each semaphore individually."""
    self.nc.sync.drain()
    popped = self.nc._tile_sem_poison_stack.pop()
    assert popped is self._sem_poison
    self.nc.all_engine_barrier()


def _emit(nc: bass.Bass):
    x_in = nc.declare_dram_parameter("xq", [B_LOC, D_IN], F32, isOutput=False)
    tx_in = nc.declare_dram_parameter("train_x", [N_TRAIN, D_IN], F32, isOutput=False)
    y_in = nc.declare_dram_parameter("yv", [N_TRAIN], F32, isOutput=False)
    wh_in = nc.declare_dram_parameter("whp", [1, D_OUT * D_IN + 1], F32, isOutput=False)
    tbl_in = nc.declare_dram_parameter("tbl", [KD2, KD2 + 1], F32, isOutput=False)
    o_out = nc.declare_dram_parameter("out", [B_LOC, D_OUT], F32, isOutput=True)

    with tile.TileContext(nc) as tc, ExitStack() as ctx:
        sb = ctx.enter_context(tc.tile_pool(name="sb", bufs=1))
        ps = ctx.enter_context(tc.tile_pool(name="ps", bufs=1, space="PSUM"))

        # --- ACT table preload (overlaps with DMAs) ---
        warm = sb.tile([1, 1], F32)
        nc.gpsimd.memset(warm[:], 0.0)
        nc.scalar.activation(warm[:], warm[:], mybir.ActivationFunctionType.Square)
        nc.scalar.activation(warm[:], warm[:], mybir.ActivationFunctionType.Exp)

        # --- input DMAs, spread across DGE queues ---
        # train_X rows n = p*64 + c  ->  XT[p, c*4 + j]   (1KB contig/partition)
        XT = sb.tile([128, NCH * D_IN], F32)
        nc.sync.dma_start(XT[:], tx_in[:, :].rearrange("(p c) d -> p (c d)", p=128))

        # W flat + h broadcast to all 128 partitions via stride-0 DMA
        Wb = sb.tile([128, D_OUT * D_IN + 1], F32)
        nc.scalar.dma_start(
            Wb[:], wh_in[:, :].broadcast_to([128, D_OUT * D_IN + 1]))

        # Y with the same n = p*64 + c mapping
        YT = sb.tile([128, NCH], F32)
        nc.sync.dma_start(YT[:], y_in[:].rearrange("(p c) -> p c", p=128))

        # queries: rows b = p*4 + c -> XQ[p, c*4 + j]
        XQ = sb.tile([128, QC * D_IN], F32)
        nc.sync.dma_start(XQ[:], x_in[:, :].rearrange("(p c) d -> p (c d)", p=128))

        # const tables: col 0 = 1/k! (78 rows: num | den), cols 1..79 = I(78)
        tblT = sb.tile([KD2, KD2 + 1], F32)
        nc.gpsimd.dma_start(tblT[:], tbl_in[:, :])

        ones_row = sb.tile([1, 128], F32)
        nc.vector.memset(ones_row[:], 1.0)

        # --- alpha = 1/h^2 per-partition columns ---
        hcol = Wb[:, 12:13]
        h2 = sb.tile([128, 1], F32)
        nc.vector.tensor_mul(h2[:], hcol, hcol)
        acol = sb.tile([128, 1], F32)
        nc.vector.reciprocal(acol[:], h2[:])
        nah = sb.tile([128, 1], F32)      # -alpha/2
        nc.vector.tensor_scalar_mul(nah[:], acol[:], -0.5)

        # --- Z[p, d*64+c] = sum_j XT[p,c,j] * W[d,j]  (DVE, (d,c) layout) ---
        PROD = sb.tile([128, D_OUT * NCH * D_IN], F32)
        xt_v = XT[:].rearrange("p (c j) -> p c j", j=D_IN)          # [128,64,4]
        xt_b = xt_v.unsqueeze(1).broadcast_to([128, D_OUT, NCH, D_IN])
        w_v = Wb[:, 0:12].rearrange("p (d j) -> p d j", j=D_IN)     # [128,3,4]
        w_b = w_v.unsqueeze(2).broadcast_to([128, D_OUT, NCH, D_IN])
        prod_v = PROD[:].rearrange("p (d c j) -> p d c j", c=NCH, j=D_IN)
        nc.vector.tensor_mul(prod_v, xt_b, w_b)
        Z = sb.tile([128, CD], F32)
        nc.vector.tensor_reduce(
            Z[:].rearrange("p (d c) -> p d c", c=NCH), prod_v,
            axis=mybir.AxisListType.X, op=mybir.AluOpType.add)

        # ZA = alpha * Z ; ZA2 = ZA^2
        ZA = sb.tile([128, CD], F32)
        nc.vector.tensor_scalar_mul(ZA[:], Z[:], acol[:, 0:1])
        ZA2 = sb.tile([128, CD], F32)
        nc.vector.tensor_mul(ZA2[:], ZA[:], ZA[:])

        # u = exp(-alpha/2 * Z^2)  (ACT)
        ZSQ = sb.tile([128, CD], F32)
        nc.scalar.activation(ZSQ[:], Z[:], mybir.ActivationFunctionType.Square)

        # V layout: col = k*CD + d*NCH + c; V_k contiguous [128, 192] blocks.
        # V0 = u (ACT writes it directly), V1 = u*ZA, V_k = V_{k-2}*ZA2 (DVE)
        V = sb.tile([128, NK * CD], F32)
        nc.scalar.activation(V[:, 0:CD], ZSQ[:],
                             mybir.ActivationFunctionType.Exp, scale=nah[:, 0:1])
        nc.vector.tensor_mul(V[:, CD : 2 * CD], V[:, 0:CD], ZA[:])
        for k in range(2, NK):
            nc.vector.tensor_mul(V[:, k * CD : (k + 1) * CD],
                                 V[:, (k - 2) * CD : (k - 1) * CD], ZA2[:])

        # --- moments ---
        # PART[:, 0:39]  = sum_c Y*V   (DVE: weighted mul then c-reduce)
        # PART[:, 39:78] = sum_c   V   (DVE c-reduce)
        PART = sb.tile([128, KD2], F32)
        v_kdc = V[:].rearrange("p (e c) -> p e c", c=NCH)          # e = (k,d)
        VY = sb.tile([128, NK * CD], F32)
        y_b = YT[:].unsqueeze(1).broadcast_to([128, KD, NCH])
        nc.vector.tensor_mul(
            VY[:].rearrange("p (e c) -> p e c", c=NCH), v_kdc, y_b)
        nc.vector.tensor_reduce(
            PART[:, 0:KD], VY[:].rearrange("p (e c) -> p e c", c=NCH),
            axis=mybir.AxisListType.X, op=mybir.AluOpType.add)
        nc.vector.tensor_reduce(
            PART[:, KD:KD2], v_kdc,
            axis=mybir.AxisListType.X, op=mybir.AluOpType.add)

        # --- single partition-reduction matmul: psum[j, 0] = sum_p PART[p, j] ---
        ones_col = sb.tile([128, 1], F32)
        nc.vector.memset(ones_col[:], 1.0)
        ps_m = ps.tile([KD2, 1], F32)
        nc.tensor.matmul(ps_m[:], PART[:], ones_col[:], start=True, stop=True)

        # scale by 1/k!
        msb = sb.tile([KD2, 1], F32)
        nc.vector.tensor_mul(msb[:], ps_m[:], tblT[:, 0:1])

        # transpose [78,1] -> [1,78], broadcast to 128 partitions
        ps_t = ps.tile([1, KD2], F32)
        nc.tensor.transpose(ps_t[:], msb[:], tblT[:, 1 : KD2 + 1])
        mt = sb.tile([1, KD2], F32)
        nc.vector.tensor_copy(mt[:], ps_t[:])
        ps_AB = ps.tile([128, KD2], F32)
        nc.tensor.matmul(ps_AB[:], ones_row[:], mt[:], start=True, stop=True)

        # --- query side (GpSimd, parallel with train side) ---
        PRODQ = sb.tile([128, QC * D_OUT * D_IN], F32)
        xq_v = XQ[:].rearrange("p (c j) -> p c j", j=D_IN)
        xq_b = xq_v.unsqueeze(2).broadcast_to([128, QC, D_OUT, D_IN])
        wq_b = w_v.unsqueeze(1).broadcast_to([128, QC, D_OUT, D_IN])
        prodq_v = PRODQ[:].rearrange("p (c d j) -> p c d j", d=D_OUT, j=D_IN)
        nc.gpsimd.tensor_mul(prodq_v, xq_b, wq_b)
        XWQ = sb.tile([128, QCD], F32)
        nc.vector.tensor_reduce(
            XWQ[:].rearrange("p (c d) -> p c d", d=D_OUT), prodq_v,
            axis=mybir.AxisListType.X, op=mybir.AluOpType.add)

        # Q layout: col = c*NK*D_OUT + k*D_OUT + d; even/odd chains via XW^2
        Q = sb.tile([128, QC * KD], F32)
        q_ckd = Q[:].rearrange("p (c k d) -> p c k d", k=NK, d=D_OUT)
        xw_cd = XWQ[:].rearrange("p (c d) -> p c d", d=D_OUT)
        nc.gpsimd.memset(q_ckd[:, :, 0, :], 1.0)
        nc.gpsimd.tensor_copy(q_ckd[:, :, 1, :], xw_cd)
        XW2 = sb.tile([128, QCD], F32)
        nc.gpsimd.tensor_mul(XW2[:], XWQ[:], XWQ[:])
        xw2_cd = XW2[:].rearrange("p (c d) -> p c d", d=D_OUT)
        for k in range(2, NK):
            nc.gpsimd.tensor_mul(q_ckd[:, :, k, :], q_ckd[:, :, k - 2, :], xw2_cd)

        # num/den = sum_k coeff[k,d] * Q[:, c, k, d]
        a_b = ps_AB[:, 0:KD].rearrange("p (k d) -> p k d", d=D_OUT) \
            .unsqueeze(1).broadcast_to([128, QC, NK, D_OUT])
        c_b = ps_AB[:, KD:KD2].rearrange("p (k d) -> p k d", d=D_OUT) \
            .unsqueeze(1).broadcast_to([128, QC, NK, D_OUT])
        TTN = sb.tile([128, QC * KD], F32)
        ttn_v = TTN[:].rearrange("p (c k d) -> p c k d", k=NK, d=D_OUT)
        nc.vector.tensor_mul(ttn_v, q_ckd, a_b)
        TTD = sb.tile([128, QC * KD], F32)
        ttd_v = TTD[:].rearrange("p (c k d) -> p c k d", k=NK, d=D_OUT)
        nc.vector.tensor_mul(ttd_v, q_ckd, c_b)

        NUMQ = sb.tile([128, QCD], F32)
        nc.vector.tensor_reduce(
            NUMQ[:].rearrange("p (c d) -> p c d", d=D_OUT),
            ttn_v.transpose([0, 1, 3, 2]),
            axis=mybir.AxisListType.X, op=mybir.AluOpType.add)
        DENQ = sb.tile([128, QCD], F32)
        nc.vector.tensor_reduce(
            DENQ[:].rearrange("p (c d) -> p c d", d=D_OUT),
            ttd_v.transpose([0, 1, 3, 2]),
            axis=mybir.AxisListType.X, op=mybir.AluOpType.add)

        RCP = sb.tile([128, QCD], F32)
        nc.vector.reciprocal(RCP[:], DENQ[:])
        OUTV = sb.tile([128, QCD], F32)
        nc.vector.tensor_mul(OUTV[:], NUMQ[:], RCP[:])

        nc.sync.dma_start(o_out[:, :].rearrange("(p c) d -> p (c d)", p=128),
                          OUTV[:])
        # WAR sentinel: overwriting OUTV forces a wait for the out-DMA's
        # completion, so the lean tail barrier needs no per-sem waits.
        nc.vector.memset(OUTV[0:1, 0:1], 0.0)
    return nc


_NC_CACHE = None


def _get_nc():
    global _NC_CACHE
    if _NC_CACHE is None:
        orig = tile.TileContext._drain_and_barrier
        tile.TileContext._drain_and_barrier = _lean_drain_and_barrier
        try:
            nc = bacc.Bacc(
                "TRN2",
                target_bir_lowering=False,
                debug=False,
                enable_asserts=True,
                num_devices=N_CORES,
            )
            _emit(nc)
            nc.finalize()
        finally:
            tile.TileContext._drain_and_barrier = orig
        _NC_CACHE = nc
    return _NC_CACHE


def _const_inputs():
    tbl = np.zeros([KD2, KD2 + 1], np.float32)
    for k in range(NK):
        tbl[k * D_OUT : (k + 1) * D_OUT, 0] = 1.0 / math.factorial(k)
        tbl[KD + k * D_OUT : KD + (k + 1) * D_OUT, 0] = 1.0 / math.factorial(k)
    tbl[:, 1 : KD2 + 1] = np.eye(KD2, dtype=np.float32)
    return tbl


def _run(x, train_X, Y, W, h, **spmd_kwargs):
    x = np.ascontiguousarray(np.asarray(x, np.float32))
    train_X = np.ascontiguousarray(np.asarray(train_X, np.float32))
    Y = np.ascontiguousarray(np.asarray(Y, np.float32))
    W = np.ascontiguousarray(np.asarray(W, np.float32))
    whp = np.concatenate(
        [W.reshape(-1), np.asarray(h, np.float32).reshape(-1)]).reshape(1, -1)
    tbl = _const_inputs()

    nc = _get_nc()
    in_maps = []
    for i in range(N_CORES):
        in_maps.append({
            "xq": x[i * B_LOC : (i + 1) * B_LOC],
            "train_x": train_X,
            "yv": Y,
            "whp": whp,
            "tbl": tbl,
        })
    return run_bass_kernel_spmd(nc, in_maps, list(range(N_CORES)), **spmd_kwargs)


def kernel(x, train_X, Y, W, h):
    res = _run(x, train_X, Y, W, h)
    out = np.concatenate([res.results[i]["out"] for i in range(N_CORES)], axis=0)
    return out.astype(np.float32)
